# revision 6
# baseline (speedup 1.0000x reference)
"""Trainium2 Bass kernel for nn_AlignmentMatrix.

Math: out[b,i,j] = ctx[b,i,:]@w1 + asp[b,j,:]@w2 + (ctx[b,i,:]*w3)@asp[b,j,:]
where w_u = cat([w1,w2,w3]).

Host-side refactor: fold everything into one batched matmul
    out[b].T = M_aug[b].T @ ctxT_aug[b]
with
    M_aug[b]    = [w3[:,None]*asp[b].T + w1[:,None] ; asp_term[b][None,:]]  (D+1, L2)
    ctxT_aug[b] = [ctx[b].T ; ones(1, L1)]                                   (D+1, L1)
    asp_term[b] = asp[b] @ w2
The device kernel is a pure streaming batched matmul in bf16 (f32 PSUM
accumulate), data-parallel over batch across 8 NeuronCores.  The host
pre-transposes ctx so the contraction dim lands on SBUF partitions with
fully-contiguous DMA rows, and the device writes out^T which the host
transposes back.
"""

import numpy as np
import ml_dtypes

# Problem shape (hardcoded per spec)
B, L1, L2, D = 64, 512, 32, 600
NCORES = 8
NB = B // NCORES          # batches per core
DA = D + 1                # augmented contraction (ones row folds asp_term)
KP = 121                  # partition chunk of contraction dim
NCH = 5                   # chunks
DP = KP * NCH             # 605, padded (pad rows are zero in M => no-op)

_CACHE = {}


def _ensure_profile_hook():
    """Register the NTFF profile hook so trace=True works under axon."""
    import sys, types
    if 'antenv.axon_hooks' in sys.modules:
        return
    try:
        from trn_agent_boot.trn_boot import _ntff_profile_via_ctypes
        hook = _ntff_profile_via_ctypes('/opt/axon/libaxon_pjrt.so')
        mod = types.ModuleType('antenv.axon_hooks')
        mod.get_axon_ntff_profile_hook = lambda: hook
        sys.modules['antenv.axon_hooks'] = mod
    except Exception:
        pass


def _build_nc():
    """Build the per-core Bass graph (identical SPMD program for all 8 cores)."""
    import concourse.bacc as bacc
    import concourse.tile as tile
    import concourse.mybir as mybir

    bf16 = mybir.dt.bfloat16
    f32 = mybir.dt.float32

    nc = bacc.Bacc(None, target_bir_lowering=False)
    ctxt_ext = nc.declare_dram_parameter("ctxt", [NB, DP, L1], bf16, isOutput=False)
    m_ext = nc.declare_dram_parameter("m", [DP, NB * L2], bf16, isOutput=False)
    out_ext = nc.declare_dram_parameter("out", [NB, L2, L1], f32, isOutput=True)

    with tile.TileContext(nc) as tc:
        import contextlib
        with contextlib.ExitStack() as ctx:
            m_pool = ctx.enter_context(tc.tile_pool(name="m_pool", bufs=1))
            ctx_pool = ctx.enter_context(tc.tile_pool(name="ctx_pool", bufs=NB))
            out_pool = ctx.enter_context(tc.tile_pool(name="out_pool", bufs=4))
            psum_pool = ctx.enter_context(
                tc.tile_pool(name="psum_pool", bufs=4, space="PSUM")
            )

            # All M matrices up front: [KP, (c, b, j)]
            m_sb = m_pool.tile([KP, NCH * NB * L2], bf16)
            nc.sync.dma_start(
                out=m_sb[:].rearrange("p (c n) -> p c n", c=NCH),
                in_=m_ext.rearrange("(c p) n -> p c n", p=KP),
            )

            # ctx tiles: one DMA per batch, [KP, (c, i)]
            ctx_tiles = []
            for b in range(NB):
                t = ctx_pool.tile([KP, NCH * L1], bf16, tag="ctx")
                nc.sync.dma_start(
                    out=t[:].rearrange("p (c i) -> p c i", c=NCH),
                    in_=ctxt_ext[b].rearrange("(c p) i -> p c i", p=KP),
                )
                ctx_tiles.append(t)

            for b in range(NB):
                psum = psum_pool.tile([L2, L1], f32, tag="psum")
                for c in range(NCH):
                    nc.tensor.matmul(
                        psum[:],
                        m_sb[:, (c * NB + b) * L2:(c * NB + b + 1) * L2],
                        ctx_tiles[b][:, c * L1:(c + 1) * L1],
                        start=(c == 0),
                        stop=(c == NCH - 1),
                    )
                o = out_pool.tile([L2, L1], f32, tag="out")
                nc.vector.tensor_copy(o[:], psum[:])
                nc.scalar.dma_start(out=out_ext[b], in_=o[:])

    nc.compile()
    return nc


def _get_nc():
    if 'nc' not in _CACHE:
        _CACHE['nc'] = _build_nc()
    return _CACHE['nc']


def _prepare_in_maps(ctx, asp, w_u):
    ctx = np.asarray(ctx, dtype=np.float32)
    asp = np.asarray(asp, dtype=np.float32)
    w = np.asarray(w_u, dtype=np.float32).reshape(-1)
    w1, w2, w3 = w[:D], w[D:2 * D], w[2 * D:]

    # ctxT_aug padded: [B, DP, L1]
    ctxt = np.empty((B, DP, L1), dtype=ml_dtypes.bfloat16)
    ctxt[:, :D, :] = ctx.transpose(0, 2, 1).astype(ml_dtypes.bfloat16)
    ctxt[:, D, :] = np.float32(1.0)
    ctxt[:, D + 1:, :] = 0

    # M_aug padded: [B, DP, L2]
    m = np.zeros((B, DP, L2), dtype=np.float32)
    m[:, :D, :] = asp.transpose(0, 2, 1) * w3[None, :, None] + w1[None, :, None]
    m[:, D, :] = asp @ w2
    m_bf = m.astype(ml_dtypes.bfloat16)

    in_maps = []
    for core in range(NCORES):
        sl = slice(core * NB, (core + 1) * NB)
        in_maps.append({
            "ctxt": np.ascontiguousarray(ctxt[sl]),
            # [DP, NB*L2] d-major so DMA rows are contiguous
            "m": np.ascontiguousarray(
                m_bf[sl].transpose(1, 0, 2).reshape(DP, NB * L2)
            ),
        })
    return in_maps


def run(inputs, trace=False, trace_kwargs=None):
    """Run the kernel on the full inputs; returns (out, BassKernelResults)."""
    from concourse import bass_utils
    from concourse.bass_utils import run_bass_kernel_spmd

    if trace:
        _ensure_profile_hook()
        bass_utils.upload_artifacts = lambda tmpdir: tmpdir

    in_maps = _prepare_in_maps(inputs["ctx"], inputs["asp"], inputs["w_u"])
    nc = _get_nc()
    res = run_bass_kernel_spmd(
        nc, in_maps, core_ids=list(range(NCORES)), trace=trace,
        **(trace_kwargs or {}),
    )
    # Gather: device wrote out^T per batch; transpose back and concat cores.
    out = np.concatenate(
        [np.asarray(res.results[i]["out"]).transpose(0, 2, 1) for i in range(NCORES)],
        axis=0,
    ).astype(np.float32)
    return out, res


def kernel(batch_size, ctx, asp, w_u):
    out, _ = run({"ctx": ctx, "asp": asp, "w_u": w_u})
    return out
